# revision 38
# baseline (speedup 1.0000x reference)
"""Distributed Bass/Trainium2 kernel for nn_AreaGNN: 3x SAGEConv(mean) +
global BatchNorm + ReLU, per-graph mean/max pooling, 3-layer MLP head.
SPMD across 8 NeuronCores; takes FULL inputs, returns FULL output [G].

See DESIGN.md. Nodes sharded 6250/core; edges assigned to the dst-owning core,
sorted by (src-half, dst-tile) and padded into 128-edge blocks with a GLOBAL
(SPMD-uniform) block schedule. Neighbor features gathered with hardware
dma_gather (f16 node table); segment sums computed on the TensorEngine via
per-block one-hot S matrices (f16, carrying 1/indeg weights) accumulated in
f32 PSUM. Halo exchange = AllGather of f16 h shards into a full node table
per layer (layer 1 reads a replicated f16 copy of x). Dense phase is
feature-major so per-feature BN stats/apply are native per-partition ops.
Global BN stats and pooling partials combined with tiny AllGathers.
"""
import numpy as np

N = 50000
E = 800000
D = 128
HID = 128
G = 64
G_FEAT = 32
EPS = 1e-5
NCORES = 8
NSH = N // NCORES           # 6250
NSH_PAD = 6272              # 49 * 128
NTILES = NSH_PAD // 128     # 49
NSH2 = NSH // 2             # 3125: rows per shard half; tables are per-half
HALF = NCORES * NSH2        # 25000 rows per gather-table half (int16 idx)
BLK = 128                   # edges per S block
CBLK = 32                   # blocks per gather chunk (4096 edges)
CH = BLK * CBLK
CPW = CH // 16
TABLE_SHARED = True         # AllGather output table addr space
SPLIT_AG = True             # issue half-0 AllGather mid-transpose-loop
NQ = 4                      # SWDGE queues used for gathers


# ---------------- host-side preprocessing -----------------------------------

def _wrap_idx(idx, ch):
    """[L] -> [L/ch, 128, ch/16] int16: element m of a chunk at (m%16, m//16),
    replicated across the eight 16-partition groups."""
    L = idx.shape[0]
    out = np.empty((L // ch, 128, ch // 16), dtype=np.int16)
    w = idx.reshape(L // ch, ch // 16, 16).transpose(0, 2, 1)
    for g in range(8):
        out[:, g * 16:(g + 1) * 16, :] = w
    return out


def _preprocess(x, edge_index, batch):
    src = np.asarray(edge_index[0], dtype=np.int64)
    dst = np.asarray(edge_index[1], dtype=np.int64)
    batch = np.asarray(batch, dtype=np.int64)

    indeg = np.bincount(dst, minlength=N)
    invdeg_all = (1.0 / np.maximum(indeg, 1.0)).astype(np.float32)

    core_of = dst // NSH
    tile_of = (dst % NSH) // 128
    # half h of the table = rows [h*NSH2, (h+1)*NSH2) of EVERY core's shard,
    # so each half can be AllGather'd as soon as those shard rows exist
    src_off = src % NSH
    half_of = (src_off >= NSH2).astype(np.int64)
    # position of src within its half table (concat of per-core half-shards)
    src_pos = (src // NSH) * NSH2 + src_off - half_of * NSH2

    # per (core, half, tile) edge lists
    counts = np.zeros((NCORES, 2, NTILES), dtype=np.int64)
    buckets = {}
    for c in range(NCORES):
        mc = core_of == c
        for h in range(2):
            mh = mc & (half_of == h)
            for t in range(NTILES):
                m = mh & (tile_of == t)
                g = src_pos[m]
                d = (dst[m] % NSH) % 128        # dst within tile
                buckets[(c, h, t)] = (g, d)
                counts[c, h, t] = len(g)

    # global block schedule: NBLK[h, t] = max over cores; half A gets >= 1
    # block per tile so the pass-A psum->agg copy always initializes agg cols
    nblk = np.ceil(counts.max(axis=0) / BLK).astype(np.int64)  # [2, NTILES]
    nblk[0] = np.maximum(nblk[0], 1)
    extra = [0, 0]
    for h in range(2):
        tot = int(nblk[h].sum())
        extra[h] = (-tot) % CBLK
    # dummy blocks appended to tile NTILES-1 of each half (S=0 -> no effect)
    sched = []   # list of (h, t) per block, in execution order
    for h in range(2):
        for t in range(NTILES):
            sched += [(h, t)] * int(nblk[h, t])
        sched += [(h, NTILES - 1)] * extra[h]
    nblk_tot = len(sched)
    nchunks = nblk_tot // CBLK
    assert nchunks * CBLK == nblk_tot
    # chunk -> table half (uniform within chunk by construction)
    chunk_half = [sched[k * CBLK][0] for k in range(nchunks)]
    for k in range(nchunks):
        assert all(sched[k * CBLK + j][0] == chunk_half[k] for j in range(CBLK))

    # per-chunk gather pieces [(col offset in chunk, num_idxs)]: gather only
    # the real (16-rounded, max-over-cores) prefix of each (h,t) run; padding
    # slots keep stale data which the S=0 columns nullify. <=1024 idx each.
    run_start = {}
    b0 = 0
    for h in range(2):
        for t in range(NTILES):
            nb = int(nblk[h, t]) + (extra[h] if t == NTILES - 1 else 0)
            run_start[(h, t)] = (b0, nb)
            b0 += nb
    r16 = {k: min(-(-int(counts[:, k[0], k[1]].max()) // 16) * 16,
                  run_start[k][1] * BLK)
           for k in run_start}
    # fixed 1024-idx grid units (the HW-validated pattern); skip units that
    # contain no real indices (pure padding - S=0 nullifies their slots)
    gather_pieces = []
    for k in range(nchunks):
        c0, c1 = k * CBLK * BLK, (k + 1) * CBLK * BLK
        iv = []
        for (h, t), (rb, nb) in run_start.items():
            if h != chunk_half[k]:
                continue
            s0, s1 = rb * BLK, rb * BLK + r16[(h, t)]
            a, b = max(s0, c0), min(s1, c1)
            if a < b:
                iv.append((a - c0, b - c0))
        pieces = []
        for q in range(0, CBLK * BLK, 1024):
            if any(a < q + 1024 and b > q for a, b in iv):
                pieces.append((q, 1024))
        gather_pieces.append(pieces)

    # per-core gather idx + S blocks following the schedule.  S holds raw
    # edge COUNTS (exact in fp8); the 1/indeg weight is applied later by a
    # diagonal-matmul fused into the per-tile transpose.
    import ml_dtypes
    gidx_cores, S_cores, giraw_cores, diag_cores = [], [], [], []
    for c in range(NCORES):
        gi = np.zeros(nblk_tot * BLK, dtype=np.int64)
        S = np.zeros((nblk_tot, BLK, 128), dtype=np.float32)
        ptr = {}
        b0 = 0
        for h in range(2):
            for t in range(NTILES):
                nb = int(nblk[h, t]) + (extra[h] if t == NTILES - 1 else 0)
                g, d = buckets[(c, h, t)]
                n = len(g)
                gi[b0 * BLK: b0 * BLK + n] = g
                rows = np.arange(n)
                S_flat = S[b0:b0 + nb].reshape(nb * BLK, 128)
                np.add.at(S_flat, (rows, d), 1.0)
                b0 += nb
        assert b0 == nblk_tot
        giraw_cores.append(gi.copy())
        gidx_cores.append(_wrap_idx(gi.astype(np.int16), CH))
        # partition-major layout [128, NBLK_TOT*128] so the per-chunk DMA has
        # long contiguous per-partition runs instead of 256B descriptors
        S_cores.append(np.ascontiguousarray(
            S.transpose(1, 0, 2).reshape(BLK, nblk_tot * 128)
        ).astype(ml_dtypes.float8_e4m3))
        dg = np.zeros((128, NTILES * 128), dtype=np.float32)
        iv = np.ones(NSH_PAD, np.float32)
        iv[:NSH] = invdeg_all[c * NSH:(c + 1) * NSH]
        for t in range(NTILES):
            dg[np.arange(128), t * 128 + np.arange(128)] = \
                iv[t * 128:(t + 1) * 128]
        diag_cores.append(dg)

    cnt_g = np.bincount(batch, minlength=G)
    inv_cnt = (1.0 / np.maximum(cnt_g, 1.0)).astype(np.float32)

    P = []
    for c in range(NCORES):
        p = np.zeros((NSH_PAD, G), dtype=np.float32)
        b = batch[c * NSH:(c + 1) * NSH]
        p[np.arange(NSH), b] = inv_cnt[b]
        P.append(p)

    NG, Smax = 0, 0
    groups_c = []
    for c in range(NCORES):
        b = batch[c * NSH:(c + 1) * NSH]
        glo, ghi = int(b.min()), int(b.max())
        groups = [(g, np.where(b == g)[0]) for g in range(glo, ghi + 1)]
        groups_c.append((glo, groups))
        NG = max(NG, ghi - glo + 1)
        Smax = max(Smax, max(len(gr) for _, gr in groups))
    S_slot = ((Smax + 127) // 128) * 128   # whole free-columns per graph row
    slot, route = [], []
    for c in range(NCORES):
        glo, groups = groups_c[c]
        sm = np.full(NG * S_slot, NSH, dtype=np.int64)   # NSH = zero dummy row
        R = np.zeros((NG, G), dtype=np.float32)
        for g, gr in groups:
            r = g - glo
            sm[r * S_slot:r * S_slot + len(gr)] = gr
            R[r, g] = 1.0
        slot.append(_wrap_idx(sm.astype(np.int16), NG * S_slot)[0])
        route.append(R)

    return dict(nblk=nblk, extra=extra, sched=sched, nblk_tot=nblk_tot,
                nchunks=nchunks, chunk_half=chunk_half,
                gather_pieces=gather_pieces,
                gidx=gidx_cores, giraw=giraw_cores, S=S_cores, P=P,
                diag=diag_cores,
                slot=slot, route=route, S_slot=S_slot, NG=NG)


# ---------------- device kernel builder --------------------------------------

def _build(nc, pre):
    import concourse.mybir as mybir
    import concourse.tile as tile

    f32 = mybir.dt.float32
    f16 = mybir.dt.float16
    f8 = mybir.dt.float8e4
    i16 = mybir.dt.int16
    NCH = pre['nchunks']
    NBLK_TOT = pre['nblk_tot']
    NG, S_slot = pre['NG'], pre['S_slot']
    NSLOT = NG * S_slot
    sched = pre['sched']

    # block index -> (start, stop) flags for its (h, t) run
    is_start = [True] * NBLK_TOT
    is_stop = [True] * NBLK_TOT
    for b in range(NBLK_TOT):
        if b > 0 and sched[b] == sched[b - 1]:
            is_start[b] = False
        if b < NBLK_TOT - 1 and sched[b] == sched[b + 1]:
            is_stop[b] = False

    # ---- I/O ----
    # layer-0 messages pre-gathered host-side into the chunk SBUF layout
    msgs0_d = nc.dram_tensor("msgs0", [NCH, 128, CBLK * D], f16,
                             kind="ExternalInput")
    xownT = nc.dram_tensor("xownT", [D, NSH_PAD], f32, kind="ExternalInput")
    gidx_d = nc.dram_tensor("gidx", [NCH, 128, CPW], i16, kind="ExternalInput")
    S_d = nc.dram_tensor("S", [BLK, NBLK_TOT * 128], f8, kind="ExternalInput")
    diag_d = nc.dram_tensor("diag", [128, NTILES * 128], f32,
                            kind="ExternalInput")
    slot_d = nc.dram_tensor("slot", [128, NSLOT // 16], i16, kind="ExternalInput")
    P_d = nc.dram_tensor("P", [NSH_PAD, G], f32, kind="ExternalInput")
    route_d = nc.dram_tensor("route", [NG, G], f32, kind="ExternalInput")
    gfT_d = nc.dram_tensor("gfT", [G_FEAT, G], f32, kind="ExternalInput")
    ident_d = nc.dram_tensor("ident", [128, 128], f32, kind="ExternalInput")
    Wl_d = [nc.dram_tensor(f"Wl{i}", [D, HID], f32, kind="ExternalInput")
            for i in range(3)]
    Wr_d = [nc.dram_tensor(f"Wr{i}", [D, HID], f32, kind="ExternalInput")
            for i in range(3)]
    gb_d = [nc.dram_tensor(f"gb{i}", [HID, 2], f32, kind="ExternalInput")
            for i in range(3)]
    W1_d = nc.dram_tensor("W1", [2 * HID + G_FEAT, HID], f32, kind="ExternalInput")
    W2_d = nc.dram_tensor("W2", [HID, HID // 2], f32, kind="ExternalInput")
    W3_d = nc.dram_tensor("W3", [HID // 2, 1], f32, kind="ExternalInput")
    bT_d = nc.dram_tensor("bT", [HID, 3], f32, kind="ExternalInput")

    out_d = nc.dram_tensor("out", [G], f32, kind="ExternalOutput")

    rg = [list(range(NCORES))]

    with tile.TileContext(nc) as tc:
        with (
            tc.tile_pool(name="sb", bufs=3) as sb,
            tc.tile_pool(name="big", bufs=2) as bigp,       # zT (3.2MB each)
            tc.tile_pool(name="agg", bufs=1) as aggp,       # agg_sb 3.2MB
            tc.tile_pool(name="big1", bufs=1) as big1,      # gmax/allp
            tc.tile_pool(name="msg", bufs=3) as msgp,       # 1MB msgs + 1MB S
            tc.tile_pool(name="idx", bufs=3) as idxp,
            tc.tile_pool(name="cst", bufs=1) as cst,
            tc.tile_pool(name="ps", bufs=2, space="PSUM") as ps,
            tc.tile_pool(name="pst", bufs=2, space="PSUM") as pst,
            tc.tile_pool(name="psa", bufs=2, space="PSUM") as psa,
            tc.tile_pool(name="psm", bufs=1, space="PSUM") as psm,
            tc.tile_pool(name="dram", bufs=1, space="DRAM") as dram,
        ):
            # ---- DRAM scratch ----
            # hbounce + gather table split per shard-half so the half-0
            # AllGather can fire mid-transpose-loop and overlap the rest
            hbounce = [[dram.tile([NSH2, D], f16, tag=f"hb{i}{h}",
                                  name=f"hb{i}{h}") for h in range(2)]
                       for i in range(2)]
            table = [[dram.tile([HALF, D], f16, tag=f"tbl{i}{h}",
                                name=f"tbl{i}{h}",
                                addr_space="Shared" if TABLE_SHARED
                                else "Local") for h in range(2)]
                     for i in range(2)]
            h3bf = dram.tile([NSH + 128, D], f16, tag="h3bf")
            stats_in = [dram.tile([D, 2], f32, tag=f"stats_in{i}",
                                  name=f"stats_in{i}") for i in range(3)]
            stats_out = [dram.tile([NCORES * D, 2], f32, tag=f"stats_out{i}",
                                   name=f"stats_out{i}", addr_space="Shared")
                         for i in range(3)]
            pool_in = dram.tile([D, 2 * G], f32, tag="pool_in")
            pool_out = dram.tile([NCORES * D, 2 * G], f32, tag="pool_out",
                                 addr_space="Shared")

            def load_const(src_ap, rows, cols, name, dt=f32):
                t = cst.tile([rows, cols], dt, tag=name)
                nc.sync.dma_start(out=t[:, :], in_=src_ap)
                return t

            ident_sb = load_const(ident_d[:, :], 128, 128, "ident")
            diag_sb = load_const(diag_d[:, :], 128, NTILES * 128, "diag")
            xT_sb = bigp.tile([128, NSH_PAD], f32, tag="zT")
            nc.sync.dma_start(out=xT_sb[:, :], in_=xownT[:, :])

            hT_prev = xT_sb

            for li in range(3):
                Wl_sb = load_const(Wl_d[li][:, :], D, HID, f"Wl{li}")
                Wr_sb = load_const(Wr_d[li][:, :], D, HID, f"Wr{li}")
                gb_sb = load_const(gb_d[li][:, :], HID, 2, f"gb{li}")

                agg_sb = aggp.tile([128, NSH_PAD], f32, tag="agg")

                # ---- gather + S-matmul aggregation ----
                acc_ps = None
                for k in range(NCH):
                    h = pre['chunk_half'][k]
                    msgs = msgp.tile([128, CBLK, D], f16, tag="msgs")
                    if li == 0:
                        # layer-0 messages were pre-gathered host-side:
                        # plain contiguous stream, no SWDGE gather needed
                        nc.sync.dma_start(
                            out=msgs[:, :, :],
                            in_=msgs0_d[k, :, :].rearrange(
                                "p (j d) -> p j d", d=D))
                    else:
                        src_tab = table[(li - 1) % 2][h][:, :]
                        gi = idxp.tile([128, CPW], i16, tag="gi")
                        nc.sync.dma_start(out=gi[:], in_=gidx_d[k, :, :])
                        # this runtime faults on dma_gather with num_idxs >
                        # 1024: <=1024-index sub-gathers, skipping per-run
                        # padding tails; one queue per chunk, rotating
                        q = k % NQ
                        for off, n in pre['gather_pieces'][k]:
                            nc.gpsimd.dma_gather(
                                msgs[:, off // 128:off // 128
                                     + (n + 127) // 128, :], src_tab,
                                gi[:, off // 16:off // 16 + n // 16], n, n, D,
                                queue_num=q)
                    S_sb = msgp.tile([128, CBLK, D], f8, tag="Ssb")
                    nc.sync.dma_start(
                        out=S_sb[:, :, :],
                        in_=S_d[:, k * CBLK * 128:(k + 1) * CBLK * 128]
                        .rearrange("p (j d) -> p j d", d=128))
                    for j in range(CBLK):
                        b = k * CBLK + j
                        h_b, t_b = sched[b]
                        if is_start[b]:
                            acc_ps = psa.tile([128, D], f32, tag="accp")
                        nc.tensor.matmul(acc_ps[:, :], S_sb[:, j, :],
                                         msgs[:, j, :],
                                         start=is_start[b], stop=is_stop[b])
                        if is_stop[b]:
                            if h_b == 0:
                                nc.scalar.copy(
                                    agg_sb[:, t_b * 128:(t_b + 1) * 128],
                                    acc_ps[:, :])
                            else:
                                nc.vector.tensor_add(
                                    agg_sb[:, t_b * 128:(t_b + 1) * 128],
                                    agg_sb[:, t_b * 128:(t_b + 1) * 128],
                                    acc_ps[:, :])

                # ---- dense phase (feature-major) ----
                zT = bigp.tile([128, NSH_PAD], f32, tag="zT")
                zsum = sb.tile([128, NTILES], f32, tag="zsum")
                zsq = sb.tile([128, NTILES], f32, tag="zsq")
                sq_scr = sb.tile([128, D], f32, tag="sqscr")
                for t in range(NTILES):
                    aT_ps = pst.tile([128, D], f32, tag="tp")
                    # transpose fused with the 1/indeg column scale:
                    # out[f, j] = sum_k agg[k, f] * diag[k, j] = agg[j, f]/deg_j
                    nc.tensor.matmul(aT_ps[:, :],
                                     agg_sb[:, t * 128:(t + 1) * 128],
                                     diag_sb[:, t * 128:(t + 1) * 128],
                                     start=True, stop=True)
                    aT_sb = sb.tile([128, D], f32, tag="aTs")
                    nc.vector.tensor_copy(aT_sb[:, :], aT_ps[:, :])
                    z_ps = ps.tile([128, D], f32, tag="z")
                    nc.tensor.matmul(z_ps[:, :], Wl_sb[:, :], aT_sb[:, :],
                                     start=True, stop=False)
                    nc.tensor.matmul(z_ps[:, :], Wr_sb[:, :],
                                     hT_prev[:, t * 128:(t + 1) * 128],
                                     start=False, stop=True)
                    nc.scalar.activation(zT[:, t * 128:(t + 1) * 128],
                                         z_ps[:, :],
                                         mybir.ActivationFunctionType.Copy,
                                         accum_out=zsum[:, t:t + 1])
                    nc.scalar.activation(sq_scr[:, :],
                                         zT[:, t * 128:(t + 1) * 128],
                                         mybir.ActivationFunctionType.Square,
                                         accum_out=zsq[:, t:t + 1])
                stat_sb = sb.tile([128, 2], f32, tag="stat")
                nc.vector.tensor_reduce(stat_sb[:, 0:1], zsum[:, :],
                                        mybir.AxisListType.X,
                                        mybir.AluOpType.add)
                nc.vector.tensor_reduce(stat_sb[:, 1:2], zsq[:, :],
                                        mybir.AxisListType.X,
                                        mybir.AluOpType.add)
                nc.sync.dma_start(out=stats_in[li][:, :], in_=stat_sb[:, :])
                nc.gpsimd.collective_compute(
                    "AllGather", mybir.AluOpType.bypass, replica_groups=rg,
                    ins=[stats_in[li].opt()], outs=[stats_out[li].opt()])
                allst = sb.tile([128, NCORES, 2], f32, tag="allst")
                nc.sync.dma_start(
                    out=allst[:, :, :],
                    in_=stats_out[li][:, :].rearrange("(c p) j -> p c j",
                                                      c=NCORES))
                tot = sb.tile([128, 2], f32, tag="tot")
                nc.vector.tensor_add(tot[:, :], allst[:, 0, :], allst[:, 1, :])
                for c in range(2, NCORES):
                    nc.vector.tensor_add(tot[:, :], tot[:, :], allst[:, c, :])
                mu = sb.tile([128, 6], f32, tag="mu")
                nc.scalar.mul(mu[:, 0:1], tot[:, 0:1], 1.0 / N)
                nc.scalar.mul(mu[:, 1:2], tot[:, 1:2], 1.0 / N)
                nc.vector.tensor_mul(mu[:, 2:3], mu[:, 0:1], mu[:, 0:1])
                nc.vector.tensor_sub(mu[:, 3:4], mu[:, 1:2], mu[:, 2:3])
                nc.vector.tensor_scalar_add(mu[:, 3:4], mu[:, 3:4], EPS)
                nc.vector.reciprocal(mu[:, 4:5], mu[:, 3:4])
                nc.scalar.sqrt(mu[:, 4:5], mu[:, 4:5])
                nc.vector.tensor_mul(mu[:, 4:5], mu[:, 4:5], gb_sb[:, 0:1])
                nc.vector.tensor_mul(mu[:, 5:6], mu[:, 0:1], mu[:, 4:5])
                nc.vector.tensor_sub(mu[:, 5:6], gb_sb[:, 1:2], mu[:, 5:6])
                # h = relu(z*s + shift), real node columns only (pads stay 0)
                for ck in range(13):
                    w = 512 if ck < 12 else NSH - 12 * 512
                    nc.scalar.activation(zT[:, ck * 512:ck * 512 + w],
                                         zT[:, ck * 512:ck * 512 + w],
                                         mybir.ActivationFunctionType.Relu,
                                         bias=mu[:, 5:6], scale=mu[:, 4:5])
                hT_prev = zT

                # ---- transpose back; ship f16 shard to AG / pooling ----
                if li < 2:
                    hb = hbounce[li % 2]
                    tb = table[li % 2]
                    for t in range(NTILES):
                        hT_ps = pst.tile([128, D], f32, tag="tp")
                        nc.tensor.transpose(hT_ps[:, :],
                                            zT[:, t * 128:(t + 1) * 128],
                                            ident_sb[:, :])
                        hbf_sb = sb.tile([128, D], f16, tag="hbf")
                        nc.vector.tensor_copy(hbf_sb[:, :], hT_ps[:, :])
                        n = 128 if t < NTILES - 1 else NSH - (NTILES - 1) * 128
                        r0, r1 = t * 128, t * 128 + n
                        if r1 <= NSH2:
                            nc.sync.dma_start(out=hb[0][r0:r1, :],
                                              in_=hbf_sb[0:n, :])
                        elif r0 >= NSH2:
                            nc.sync.dma_start(out=hb[1][r0 - NSH2:r1 - NSH2, :],
                                              in_=hbf_sb[0:n, :])
                        else:
                            nc.sync.dma_start(out=hb[0][r0:NSH2, :],
                                              in_=hbf_sb[0:NSH2 - r0, :])
                            nc.sync.dma_start(out=hb[1][0:r1 - NSH2, :],
                                              in_=hbf_sb[NSH2 - r0:n, :])
                        if SPLIT_AG and r1 >= NSH2 and r0 < NSH2:
                            # half 0 complete: AllGather it now, overlapping
                            # the remaining tiles' transposes + half-1 AG
                            nc.gpsimd.collective_compute(
                                "AllGather", mybir.AluOpType.bypass,
                                replica_groups=rg,
                                ins=[hb[0].opt()], outs=[tb[0].opt()])
                    if not SPLIT_AG:
                        nc.gpsimd.collective_compute(
                            "AllGather", mybir.AluOpType.bypass,
                            replica_groups=rg,
                            ins=[hb[0].opt()], outs=[tb[0].opt()])
                    nc.gpsimd.collective_compute(
                        "AllGather", mybir.AluOpType.bypass, replica_groups=rg,
                        ins=[hb[1].opt()], outs=[tb[1].opt()])
                else:
                    meanT_ps = psm.tile([128, G], f32, tag="meanT")
                    for t in range(NTILES):
                        hT_ps = pst.tile([128, D], f32, tag="tp")
                        nc.tensor.transpose(hT_ps[:, :],
                                            zT[:, t * 128:(t + 1) * 128],
                                            ident_sb[:, :])
                        h3_sb = sb.tile([128, D], f32, tag="h3")
                        nc.vector.tensor_copy(h3_sb[:, :], hT_ps[:, :])
                        P_sb = sb.tile([128, G], f32, tag="P")
                        nc.sync.dma_start(out=P_sb[:, :],
                                          in_=P_d[t * 128:(t + 1) * 128, :])
                        nc.tensor.matmul(meanT_ps[:, :], h3_sb[:, :],
                                         P_sb[:, :],
                                         start=(t == 0),
                                         stop=(t == NTILES - 1))
                        hbf_sb = sb.tile([128, D], f16, tag="hbf")
                        nc.vector.tensor_copy(hbf_sb[:, :], h3_sb[:, :])
                        nc.sync.dma_start(out=h3bf[t * 128:(t + 1) * 128, :],
                                          in_=hbf_sb[:, :])
                    zrow = sb.tile([1, D], f16, tag="zrow")
                    nc.vector.memset(zrow[:, :], 0.0)
                    nc.sync.dma_start(out=h3bf[NSH:NSH + 1, :], in_=zrow[:, :])

            # ---- max pool: transpose-gather + segmented max + route ----
            slot_sb = cst.tile([128, NSLOT // 16], i16, tag="slot")
            nc.sync.dma_start(out=slot_sb[:, :], in_=slot_d[:, :])
            # gather the slot grid node-major (transpose gather is broken on
            # this runtime); graph row r occupies free-cols [r*SC, (r+1)*SC)
            SC = S_slot // 128
            gmax = big1.tile([128, NSLOT // 128, D], f16, tag="gmax")
            # 1024-index sub-gathers (runtime limit); NSLOT is a multiple of
            # 128 but not necessarily 1024 - last sub-gather may be shorter
            for gqi, g0 in enumerate(range(0, NSLOT, 1024)):
                g1 = min(g0 + 1024, NSLOT)
                nc.gpsimd.dma_gather(gmax[:, g0 // 128:g1 // 128, :],
                                     h3bf[0:NSH + 128, :],
                                     slot_sb[:, (g0 // 16):(g1 // 16)],
                                     g1 - g0, g1 - g0, D,
                                     queue_num=gqi % NQ)
            mloc_f = sb.tile([128, NG], f32, tag="mlocf")
            for r in range(NG):
                # max over this graph's SC columns, per (slot-partition, feat)
                red1 = sb.tile([128, D], f32, tag="red1")
                nc.vector.tensor_reduce(
                    red1[:, :],
                    gmax[:, r * SC:(r + 1) * SC, :].rearrange("p c f -> p f c"),
                    mybir.AxisListType.X, mybir.AluOpType.max)
                r1T_ps = pst.tile([128, D], f32, tag="tp", name="r1T")
                nc.tensor.transpose(r1T_ps[:, :], red1[:, :], ident_sb[:, :])
                r1T_sb = sb.tile([128, D], f32, tag="r1Ts")
                nc.vector.tensor_copy(r1T_sb[:, :], r1T_ps[:, :])
                nc.vector.tensor_reduce(mloc_f[:, r:r + 1], r1T_sb[:, :],
                                        mybir.AxisListType.X,
                                        mybir.AluOpType.max)
            mlocT_ps = pst.tile([NG, 128], f32, tag="tp")
            nc.tensor.transpose(mlocT_ps[:, :], mloc_f[:, :], ident_sb[:, :])
            mlocT_sb = sb.tile([NG, 128], f32, tag="mlocTs")
            nc.vector.tensor_copy(mlocT_sb[:, :], mlocT_ps[:, :])
            route_sb = cst.tile([NG, G], f32, tag="route")
            nc.sync.dma_start(out=route_sb[:, :], in_=route_d[:, :])
            maxT_ps = psm.tile([128, G], f32, tag="tail")
            nc.tensor.matmul(maxT_ps[:, :], mlocT_sb[:, :], route_sb[:, :],
                             start=True, stop=True)

            # ---- pool partial exchange ----
            pool_sb = sb.tile([128, 2 * G], f32, tag="poolp")
            nc.vector.tensor_copy(pool_sb[:, 0:G], meanT_ps[:, :])
            nc.vector.tensor_copy(pool_sb[:, G:2 * G], maxT_ps[:, :])
            nc.sync.dma_start(out=pool_in[:, :], in_=pool_sb[:, :])
            nc.gpsimd.collective_compute(
                "AllGather", mybir.AluOpType.bypass, replica_groups=rg,
                ins=[pool_in.opt()], outs=[pool_out.opt()])
            allp = big1.tile([128, NCORES, 2 * G], f32, tag="allp")
            nc.sync.dma_start(
                out=allp[:, :, :],
                in_=pool_out[:, :].rearrange("(c p) j -> p c j", c=NCORES))
            meanTot = sb.tile([128, G], f32, tag="meanTot")
            maxTot = sb.tile([128, G], f32, tag="maxTot")
            nc.vector.tensor_add(meanTot[:, :], allp[:, 0, 0:G],
                                 allp[:, 1, 0:G])
            nc.vector.tensor_max(maxTot[:, :], allp[:, 0, G:2 * G],
                                 allp[:, 1, G:2 * G])
            for c in range(2, NCORES):
                nc.vector.tensor_add(meanTot[:, :], meanTot[:, :],
                                     allp[:, c, 0:G])
                nc.vector.tensor_max(maxTot[:, :], maxTot[:, :],
                                     allp[:, c, G:2 * G])

            # ---- head (feature-major) ----
            W1a_sb = load_const(W1_d[0:HID, :], HID, HID, "W1a")
            W1b_sb = load_const(W1_d[HID:2 * HID, :], HID, HID, "W1b")
            W1c_sb = load_const(W1_d[2 * HID:2 * HID + G_FEAT, :], G_FEAT,
                                HID, "W1c")
            W2_sb = load_const(W2_d[:, :], HID, HID // 2, "W2")
            W3_sb = load_const(W3_d[:, :], HID // 2, 1, "W3")
            bT_sb = load_const(bT_d[:, :], HID, 3, "bT")
            gfT_sb = load_const(gfT_d[:, :], G_FEAT, G, "gfT")

            m1_ps = psm.tile([HID, G], f32, tag="tail")
            nc.tensor.matmul(m1_ps[:, :], W1a_sb[:, :], meanTot[:, :],
                             start=True, stop=False)
            nc.tensor.matmul(m1_ps[:, :], W1b_sb[:, :], maxTot[:, :],
                             start=False, stop=False)
            nc.tensor.matmul(m1_ps[:, :], W1c_sb[:, :],
                             gfT_sb[:, :], start=False, stop=True)
            m1_sb = sb.tile([HID, G], f32, tag="m1s")
            nc.scalar.activation(m1_sb[:, :], m1_ps[:, :],
                                 mybir.ActivationFunctionType.Relu,
                                 bias=bT_sb[:, 0:1])
            m2_ps = psm.tile([HID // 2, G], f32, tag="tail")
            nc.tensor.matmul(m2_ps[:, :], W2_sb[:, :], m1_sb[:, :],
                             start=True, stop=True)
            m2_sb = sb.tile([HID // 2, G], f32, tag="m2s")
            nc.scalar.activation(m2_sb[:, :], m2_ps[:, :],
                                 mybir.ActivationFunctionType.Relu,
                                 bias=bT_sb[0:HID // 2, 1:2])
            m3_ps = psm.tile([1, G], f32, tag="tail")
            nc.tensor.matmul(m3_ps[:, :], W3_sb[:, :], m2_sb[:, :],
                             start=True, stop=True)
            m3_sb = sb.tile([1, G], f32, tag="m3s")
            nc.scalar.copy(m3_sb[:, :], m3_ps[:, :])
            nc.vector.tensor_scalar_add(m3_sb[:, :], m3_sb[:, :],
                                        bT_sb[0:1, 2:3])
            nc.sync.dma_start(out=out_d[:].rearrange("(o g) -> o g", o=1),
                              in_=m3_sb[:, :])
    return nc


# ---------------- public entry ------------------------------------------------

def build_in_maps(x, edge_index, batch, g_feats, params, pre):
    x = np.asarray(x, dtype=np.float32)
    g_feats = np.asarray(g_feats, dtype=np.float32)

    bT = np.zeros((HID, 3), np.float32)
    bT[:, 0] = np.asarray(params['b1'], np.float32)
    bT[:HID // 2, 1] = np.asarray(params['b2'], np.float32)
    bT[0, 2] = np.asarray(params['b3'], np.float32).reshape(-1)[0]

    common = {
        "ident": np.eye(128, dtype=np.float32),
        "gfT": np.ascontiguousarray(g_feats.T),
        "W1": np.asarray(params['W1'], np.float32),
        "W2": np.asarray(params['W2'], np.float32),
        "W3": np.asarray(params['W3'], np.float32),
        "bT": bT,
    }
    for i in range(3):
        common[f"Wl{i}"] = np.asarray(params[f'Wl{i}'], np.float32)
        common[f"Wr{i}"] = np.asarray(params[f'Wr{i}'], np.float32)
        gb = np.zeros((HID, 2), np.float32)
        gb[:, 0] = np.asarray(params[f'gamma{i}'], np.float32)
        gb[:, 1] = np.asarray(params[f'beta{i}'], np.float32)
        common[f"gb{i}"] = gb

    x16 = x.astype(np.float16)
    NCH = pre['nchunks']
    chunk_half = np.asarray(pre['chunk_half'], dtype=np.int64)
    in_maps = []
    for c in range(NCORES):
        xo = np.zeros((NSH_PAD, D), np.float32)
        xo[:NSH] = x[c * NSH:(c + 1) * NSH]
        # pre-gather layer-0 messages into the exact chunk SBUF layout:
        # slot s of chunk k -> partition s%128, free block s//128
        gpos = pre['giraw'][c].reshape(NCH, CH)
        gi_abs = ((gpos // NSH2) * NSH + chunk_half[:, None] * NSH2
                  + gpos % NSH2).reshape(-1)
        msgs0 = x16[gi_abs].reshape(NCH, CBLK, BLK, D).transpose(0, 2, 1, 3)
        msgs0 = np.ascontiguousarray(msgs0.reshape(NCH, 128, CBLK * D))
        m = dict(common)
        m.update({
            "xownT": np.ascontiguousarray(xo.T),
            "msgs0": msgs0,
            "gidx": pre['gidx'][c],
            "S": pre['S'][c],
            "diag": pre['diag'][c],
            "slot": pre['slot'][c],
            "P": pre['P'][c],
            "route": pre['route'][c],
        })
        in_maps.append(m)
    return in_maps


def build_nc(pre):
    import os
    import concourse.bacc as bacc
    nc = bacc.Bacc(None, target_bir_lowering=False, debug=False,
                   num_devices=NCORES, num_swdge_queues=4,
                   detect_race_conditions=os.environ.get(
                       "KERNEL_NO_RACE_CHECK") != "1")
    nc = _build(nc, pre)
    nc.compile()
    return nc


def kernel(x, edge_index, batch, g_feats,
           Wl0, bl0, Wr0, gamma0, beta0,
           Wl1, bl1, Wr1, gamma1, beta1,
           Wl2, bl2, Wr2, gamma2, beta2,
           W1, b1, W2, b2, W3, b3):
    # bl{i} cancels inside BatchNorm (constant pre-BN shift), so it is unused.
    from concourse.bass_utils import run_bass_kernel_spmd

    params = dict(Wl0=Wl0, Wr0=Wr0, gamma0=gamma0, beta0=beta0,
                  Wl1=Wl1, Wr1=Wr1, gamma1=gamma1, beta1=beta1,
                  Wl2=Wl2, Wr2=Wr2, gamma2=gamma2, beta2=beta2,
                  W1=W1, b1=b1, W2=W2, b2=b2, W3=W3, b3=b3)
    pre = _preprocess(x, edge_index, batch)
    nc = build_nc(pre)
    in_maps = build_in_maps(x, edge_index, batch, g_feats, params, pre)
    res = run_bass_kernel_spmd(nc, in_maps, list(range(NCORES)))
    return np.asarray(res.results[0]["out"], dtype=np.float32)

